# revision 10
# baseline (speedup 1.0000x reference)
"""AFT-Full attention on 8 TRN2 NeuronCores (Bass/Tile, no collectives).

Reference math (B=2, TQ=TKV=512, DIM=512, HID=128, BDIM=128):
    qh  = q @ qW_w.T + qW_b
    k   = kv @ kW_w.T + kW_b
    v   = kv @ vW_w.T + vW_b
    wb  = w_bias_u @ w_bias_v                       # (TQ, TKV)
    A   = exp(k[:,None] + wb[None,:,:,None])        # (B,TQ,TKV,HID)
    out = sigmoid(qh) * (sum_s A*v / sum_s A)

Factorization: exp(k + wb) = exp(k) * exp(wb) collapses the giant A
intermediate into plain matmuls:
    num[t,h] = sum_s exp(wb[t,s]) * (exp(k0[s,h]) * v0[s,h])
    den[t,h] = sum_s exp(wb[t,s]) *  exp(k0[s,h])
The k-projection bias cancels exactly in num/den; the v bias is a pure
per-h additive term:  out = sigmoid(qh) * (num0/den0 + vW_b).

Sharding: the 1024 flattened (b, t) query rows split into 8 blocks of 128 —
core i handles batch b=i//4, queries t in [128*(i%4), 128*(i%4)+128).
Each core only needs kv[b], so no collectives (their ~7us latency floor
exceeds this whole kernel).

Host-side packing gives the device natural matmul layouts (contraction on
partitions, zero on-device transposes) AND DMA-friendly lines: each DRAM
slab is laid out exactly as its SBUF tile, so every partition reads one
contiguous 4-6.5KB run per slab (big descriptors -> near line-rate DMA).

    slab1: [kvWT(4x256) | wbv(512) | uT(128)]          (128, 1664)
    slab2: kv s-half 0, pre-tiled [dc, 256]            (128, 1024)
    slab3: kv s-half 1, pre-tiled [dc, 256]            (128, 1024)
    slab4: [qWT(4x128) | qT(4x128) | qb | vb]          (128, 1026)

Matmuls run in bf16 (f32 PSUM accumulate); DMA stays f32; casts are split
across DVE and GpSimd so the two chains chase the DMA slabs in parallel.
"""

import numpy as np

import concourse.bass as bass
import concourse.mybir as mybir
import concourse.tile as tile
from concourse import bacc
from concourse.bass_utils import run_bass_kernel_spmd

B, TQ, TKV, DIM, HID, BDIM = 2, 512, 512, 512, 128, 128
N_CORES = 8
R = (B * TQ) // N_CORES  # 128 query rows per core
P = 128
DC = DIM // P  # 4 contraction chunks for d
SC = TKV // P  # 4 contraction chunks for s
F32 = mybir.dt.float32
BF16 = mybir.dt.bfloat16
ACT = mybir.ActivationFunctionType
N_WARMUP = 0

S1 = DC * 2 * HID + TKV + BDIM  # 1664: kvWT | wbv | uT
S2 = DC * (TKV // 2)  # 1024: kv s-half 0
S3 = DC * (TKV // 2)  # 1024: kv s-half 1
S4 = DC * HID + DC * R + 2  # 1026: qWT | qT | qb | vb
O_WBV = DC * 2 * HID  # 1024 within slab1
O_UT = O_WBV + TKV  # 1536 within slab1
O_QT = DC * HID  # 512 within slab4


def _build():
    F32R = mybir.dt.float32r
    nc = bacc.Bacc(None)
    s1 = nc.declare_dram_parameter("s1", [P, S1], F32R, isOutput=False)
    s2 = nc.declare_dram_parameter("s2", [P, S2], F32R, isOutput=False)
    s3 = nc.declare_dram_parameter("s3", [P, S3], F32R, isOutput=False)
    s4 = nc.declare_dram_parameter("s4", [P, S4], F32R, isOutput=False)
    out = nc.declare_dram_parameter("out", [HID, R], F32, isOutput=True)

    with tile.TileContext(nc) as tc:
        with (
            tc.tile_pool(name="persist", bufs=1) as persist,
            tc.tile_pool(name="psum", bufs=2, space="PSUM") as psum,
            tc.tile_pool(name="psum1", bufs=1, space="PSUM") as psum1,
        ):
            # ---- slab DMAs (sync HWDGE ring, FIFO order = priority) ----
            m1 = persist.tile([P, S1], F32R, tag="m1")
            nc.sync.dma_start(out=m1[:], in_=s1[:])
            m2 = persist.tile([P, S2], F32R, tag="m2")
            nc.sync.dma_start(out=m2[:], in_=s2[:])
            m3 = persist.tile([P, S3], F32R, tag="m3")
            nc.sync.dma_start(out=m3[:], in_=s3[:])
            m4 = persist.tile([P, S4], F32R, tag="m4")
            nc.sync.dma_start(out=m4[:], in_=s4[:])

            # ---- no casts: f32 slabs feed the PE directly as float32r.
            # fp32r matmuls with moving dim >= 256 run at full (bf16) rate;
            # the narrow wbias/qh matmuls pay 2-4x on only 128 cols each.
            kvW = lambda dc: m1[:, dc * 2 * HID : (dc + 1) * 2 * HID]
            wbv = lambda sc: m1[:, O_WBV + sc * P : O_WBV + (sc + 1) * P]
            uTv = m1[:, O_UT : O_UT + R]
            # kv s-chunk sc, d-chunk dc  (sc 0..1 in slab2, 2..3 in slab3)
            kv = lambda sc, dc: (m2 if sc < 2 else m3)[
                :, dc * 256 + (sc % 2) * P : dc * 256 + (sc % 2) * P + P
            ]
            qWT = lambda dc: m4[:, dc * HID : (dc + 1) * HID]
            qTv = lambda dc: m4[:, O_QT + dc * R : O_QT + (dc + 1) * R]
            qb = m4[:, S4 - 2 : S4 - 1].bitcast(F32)
            vb = m4[:, S4 - 1 : S4].bitcast(F32)

            # ---- PE warmup: the tensor engine clock ramps 0.6->1.2->2.4GHz
            # with ~3us of sustained work; dummy matmuls during the DMA
            # stream mean the real matmuls run at full clock ----
            warm_sb = persist.tile([P, 256], BF16, tag="warm_sb")
            nc.vector.memset(warm_sb[:], 0.0)
            pwm = psum1.tile([P, 256], F32, tag="pwm")
            for _ in range(N_WARMUP):
                nc.tensor.matmul(pwm[:], lhsT=warm_sb[:, :P], rhs=warm_sb[:])

            # ---- expwbT (s,t): lhsT = wbv chunk, rhs = uT ----
            wT_bf = persist.tile([P, SC, R], BF16, tag="wT_bf")
            for sc in range(SC):
                pw = psum.tile([P, R], F32, tag="pw")
                nc.tensor.matmul(pw[:], lhsT=wbv(sc), rhs=uTv)
                nc.scalar.activation(wT_bf[:, sc, :], pw[:], ACT.Exp)

            # ---- k/v projections -> ek=exp(k0), ekv=ek*v0  (s,h), with the
            # den/num accumulations (h,t) interleaved per chunk so only the
            # last chunk's matmuls trail the final kv cast ----
            ek_bf = persist.tile([P, SC, HID], BF16, tag="ek_bf")
            ekv_bf = persist.tile([P, SC, HID], BF16, tag="ekv_bf")
            pd = psum1.tile([P, R], F32, tag="pd")
            pn = psum1.tile([P, R], F32, tag="pn")
            for sc in range(SC):
                pkv = psum.tile([P, 2 * HID], F32, tag="pkv")
                for dc in range(DC):
                    nc.tensor.matmul(
                        pkv[:],
                        lhsT=kv(sc, dc),
                        rhs=kvW(dc),
                        start=(dc == 0),
                        stop=(dc == DC - 1),
                    )
                nc.scalar.activation(ek_bf[:, sc, :], pkv[:, :HID], ACT.Exp)
                nc.vector.tensor_mul(ekv_bf[:, sc, :], ek_bf[:, sc, :], pkv[:, HID:])
                nc.tensor.matmul(
                    pd[:],
                    lhsT=ek_bf[:, sc, :],
                    rhs=wT_bf[:, sc, :],
                    start=(sc == 0),
                    stop=(sc == SC - 1),
                )
                nc.tensor.matmul(
                    pn[:],
                    lhsT=ekv_bf[:, sc, :],
                    rhs=wT_bf[:, sc, :],
                    start=(sc == 0),
                    stop=(sc == SC - 1),
                )

            # ---- qhT (h,t); sigmoid via exp so ACT never switches tables:
            # sigmoid(qh) = 1/(1+e) with e = exp(-(qh + qW_b))  (host sends -qW_b)
            pq = psum1.tile([P, R], F32, tag="pq")
            for dc in range(DC):
                nc.tensor.matmul(
                    pq[:],
                    lhsT=qWT(dc),
                    rhs=qTv(dc),
                    start=(dc == 0),
                    stop=(dc == DC - 1),
                )
            e_sb = persist.tile([P, R], F32, tag="e_sb")
            nc.scalar.activation(e_sb[:], pq[:], ACT.Exp, bias=qb, scale=-1.0)

            # ---- out = (num + vb*den) / ((1+e)*den) ----
            # vb*den on ACT (Copy with per-partition scale) so no DVE op reads
            # two PSUM tensors at once.
            vbd_sb = persist.tile([P, R], F32, tag="vbd_sb")
            nc.scalar.mul(vbd_sb[:], pd[:], vb)
            t1_sb = persist.tile([P, R], F32, tag="t1_sb")
            nc.vector.scalar_tensor_tensor(
                t1_sb[:], e_sb[:], 1.0, pd[:], mybir.AluOpType.add, mybir.AluOpType.mult
            )
            t2_sb = persist.tile([P, R], F32, tag="t2_sb")
            nc.vector.tensor_add(t2_sb[:], vbd_sb[:], pn[:])
            rec_sb = persist.tile([P, R], F32, tag="rec_sb")
            nc.vector.reciprocal_approx_fast(rec_sb[:], t1_sb[:])
            res_sb = persist.tile([P, R], F32, tag="res_sb")
            nc.vector.tensor_mul(res_sb[:], t2_sb[:], rec_sb[:])
            nc.sync.dma_start(out=out[:], in_=res_sb[:])

    nc.finalize()
    return nc


_NC_CACHE = None


def _get_nc():
    global _NC_CACHE
    if _NC_CACHE is None:
        _NC_CACHE = _build()
    return _NC_CACHE


def _make_in_maps(q, kv, qW_w, qW_b, kW_w, kW_b, vW_w, vW_b, w_bias_u, w_bias_v):
    f = lambda a: np.ascontiguousarray(np.asarray(a, dtype=np.float32))
    q, kv = f(q), f(kv)
    kvW = np.concatenate([np.asarray(kW_w), np.asarray(vW_w)], axis=0)  # (2H, DIM)
    # kvWT tiled (P, DC, 2H): [p, dc, n] = kvW[n, dc*P+p]
    kvWT_t = np.transpose(kvW.reshape(2 * HID, DC, P), (2, 1, 0))  # (P, DC, 2H)
    qWT_t = np.transpose(np.asarray(qW_w).reshape(HID, DC, P), (2, 1, 0))  # (P,DC,H)
    wbv = np.asarray(w_bias_v)  # (BDIM, TKV)
    u = np.asarray(w_bias_u)
    qf = q.reshape(B * TQ, DIM)
    # kv[b] tiled (P, 2, DC, TKV//2): [p, sh, dc, sw] = kv[b, sh*256+sw, dc*P+p]
    kv_t = [
        np.transpose(kv[b].reshape(2, TKV // 2, DC, P), (3, 0, 2, 1)) for b in range(B)
    ]
    slab1_shared = np.concatenate(
        [kvWT_t.reshape(P, -1), wbv, np.zeros((P, R), np.float32)], axis=1
    )
    in_maps = []
    for i in range(N_CORES):
        b = i // (N_CORES // B)
        t0 = (i % (N_CORES // B)) * R
        s1 = slab1_shared.copy()
        s1[:, O_UT : O_UT + R] = u[t0 : t0 + R].T  # (BDIM, R)
        # qT tiled: [p, dc, t] = qf[i*R + t, dc*P+p]
        qT_t = np.transpose(
            qf[i * R : (i + 1) * R].reshape(R, DC, P), (2, 1, 0)
        )  # (P, DC, R)
        s4 = np.concatenate(
            [
                qWT_t.reshape(P, -1),
                qT_t.reshape(P, -1),
                -np.asarray(qW_b, np.float32).reshape(P, 1),
                np.asarray(vW_b, np.float32).reshape(P, 1),
            ],
            axis=1,
        )
        in_maps.append(
            {
                "s1": f(s1),
                "s2": f(kv_t[b][:, 0].reshape(P, -1)),
                "s3": f(kv_t[b][:, 1].reshape(P, -1)),
                "s4": f(s4),
            }
        )
    return in_maps


def _run(in_maps, trace=False):
    nc = _get_nc()
    return run_bass_kernel_spmd(
        nc, in_maps, core_ids=list(range(N_CORES)), trace=trace
    )


def kernel(**inputs) -> np.ndarray:
    in_maps = _make_in_maps(**inputs)
    res = _run(in_maps)
    out = np.empty((B * TQ, HID), dtype=np.float32)
    for i in range(N_CORES):
        out[i * R : (i + 1) * R] = res.results[i]["out"].T
    return out.reshape(B, TQ, HID)


# revision 11
# speedup vs baseline: 1.0106x; 1.0106x over previous
"""AFT-Full attention on 8 TRN2 NeuronCores (Bass/Tile, no collectives).

Reference math (B=2, TQ=TKV=512, DIM=512, HID=128, BDIM=128):
    qh  = q @ qW_w.T + qW_b
    k   = kv @ kW_w.T + kW_b
    v   = kv @ vW_w.T + vW_b
    wb  = w_bias_u @ w_bias_v                       # (TQ, TKV)
    A   = exp(k[:,None] + wb[None,:,:,None])        # (B,TQ,TKV,HID)
    out = sigmoid(qh) * (sum_s A*v / sum_s A)

Factorization: exp(k + wb) = exp(k) * exp(wb) collapses the giant A
intermediate into plain matmuls:
    num[t,h] = sum_s exp(wb[t,s]) * (exp(k0[s,h]) * v0[s,h])
    den[t,h] = sum_s exp(wb[t,s]) *  exp(k0[s,h])
The k-projection bias cancels exactly in num/den; the v bias is a pure
per-h additive term:  out = sigmoid(qh) * (num0/den0 + vW_b).

Sharding: the 1024 flattened (b, t) query rows split into 8 blocks of 128 —
core i handles batch b=i//4, queries t in [128*(i%4), 128*(i%4)+128).
Each core only needs kv[b], so no collectives (their ~7us latency floor
exceeds this whole kernel).

Host-side packing gives the device natural matmul layouts (contraction on
partitions, zero on-device transposes) AND DMA-friendly lines: each DRAM
slab is laid out exactly as its SBUF tile, so every partition reads one
contiguous 4-6.5KB run per slab (big descriptors -> near line-rate DMA).

    slab1: [kvWT(4x256) | wbv(512) | uT(128)]          (128, 1664)
    slab2: kv s-half 0, pre-tiled [dc, 256]            (128, 1024)
    slab3: kv s-half 1, pre-tiled [dc, 256]            (128, 1024)
    slab4: [qWT(4x128) | qT(4x128) | qb | vb]          (128, 1026)

Matmuls run in bf16 (f32 PSUM accumulate); DMA stays f32; casts are split
across DVE and GpSimd so the two chains chase the DMA slabs in parallel.
"""

import numpy as np

import concourse.bass as bass
import concourse.mybir as mybir
import concourse.tile as tile
from concourse import bacc
from concourse.bass_utils import run_bass_kernel_spmd

B, TQ, TKV, DIM, HID, BDIM = 2, 512, 512, 512, 128, 128
N_CORES = 8
R = (B * TQ) // N_CORES  # 128 query rows per core
P = 128
DC = DIM // P  # 4 contraction chunks for d
SC = TKV // P  # 4 contraction chunks for s
F32 = mybir.dt.float32
BF16 = mybir.dt.bfloat16
ACT = mybir.ActivationFunctionType
N_WARMUP = 24

S1 = DC * 2 * HID + TKV + BDIM  # 1664: kvWT | wbv | uT
S2 = DC * (TKV // 2)  # 1024: kv s-half 0
S3 = DC * (TKV // 2)  # 1024: kv s-half 1
S4 = DC * HID + DC * R + 2  # 1026: qWT | qT | qb | vb
O_WBV = DC * 2 * HID  # 1024 within slab1
O_UT = O_WBV + TKV  # 1536 within slab1
O_QT = DC * HID  # 512 within slab4


def _build():
    F32R = mybir.dt.float32r
    nc = bacc.Bacc(None)
    s1 = nc.declare_dram_parameter("s1", [P, S1], F32R, isOutput=False)
    s2 = nc.declare_dram_parameter("s2", [P, S2], F32R, isOutput=False)
    s3 = nc.declare_dram_parameter("s3", [P, S3], F32R, isOutput=False)
    s4 = nc.declare_dram_parameter("s4", [P, S4], F32R, isOutput=False)
    out = nc.declare_dram_parameter("out", [HID, R], F32, isOutput=True)

    with tile.TileContext(nc) as tc:
        with (
            tc.tile_pool(name="persist", bufs=1) as persist,
            tc.tile_pool(name="psum", bufs=2, space="PSUM") as psum,
            tc.tile_pool(name="psum1", bufs=1, space="PSUM") as psum1,
        ):
            # ---- slab DMAs (sync HWDGE ring, FIFO order = priority) ----
            m1 = persist.tile([P, S1], F32R, tag="m1")
            nc.sync.dma_start(out=m1[:], in_=s1[:])
            m2 = persist.tile([P, S2], F32R, tag="m2")
            nc.sync.dma_start(out=m2[:], in_=s2[:])
            m3 = persist.tile([P, S3], F32R, tag="m3")
            nc.sync.dma_start(out=m3[:], in_=s3[:])
            m4 = persist.tile([P, S4], F32R, tag="m4")
            nc.sync.dma_start(out=m4[:], in_=s4[:])

            # ---- no casts: f32 slabs feed the PE directly as float32r.
            # fp32r matmuls with moving dim >= 256 run at full (bf16) rate;
            # the narrow wbias/qh matmuls pay 2-4x on only 128 cols each.
            kvW = lambda dc: m1[:, dc * 2 * HID : (dc + 1) * 2 * HID]
            wbv = lambda sc: m1[:, O_WBV + sc * P : O_WBV + (sc + 1) * P]
            uTv = m1[:, O_UT : O_UT + R]
            # kv s-chunk sc, d-chunk dc  (sc 0..1 in slab2, 2..3 in slab3)
            kv = lambda sc, dc: (m2 if sc < 2 else m3)[
                :, dc * 256 + (sc % 2) * P : dc * 256 + (sc % 2) * P + P
            ]
            qWT = lambda dc: m4[:, dc * HID : (dc + 1) * HID]
            qTv = lambda dc: m4[:, O_QT + dc * R : O_QT + (dc + 1) * R]
            qb = m4[:, S4 - 2 : S4 - 1].bitcast(F32)
            vb = m4[:, S4 - 1 : S4].bitcast(F32)

            # ---- PE warmup: the tensor engine clock ramps 0.6->1.2->2.4GHz
            # with ~3us of sustained work; dummy matmuls during the DMA
            # stream mean the real matmuls run at full clock ----
            warm_sb = persist.tile([P, 256], BF16, tag="warm_sb")
            nc.vector.memset(warm_sb[:], 0.0)
            pwm = psum1.tile([P, 256], F32, tag="pwm")
            for _ in range(N_WARMUP):
                nc.tensor.matmul(pwm[:], lhsT=warm_sb[:, :P], rhs=warm_sb[:])

            # ---- expwbT (s,t): lhsT = wbv chunk, rhs = uT ----
            wT_bf = persist.tile([P, SC, R], BF16, tag="wT_bf")
            for sc in range(SC):
                pw = psum.tile([P, R], F32, tag="pw")
                nc.tensor.matmul(pw[:], lhsT=wbv(sc), rhs=uTv)
                nc.scalar.activation(wT_bf[:, sc, :], pw[:], ACT.Exp)

            # ---- k/v projections -> ek=exp(k0), ekv=ek*v0  (s,h), with the
            # den/num accumulations (h,t) interleaved per chunk so only the
            # last chunk's matmuls trail the final kv cast ----
            ek_bf = persist.tile([P, SC, HID], BF16, tag="ek_bf")
            ekv_bf = persist.tile([P, SC, HID], BF16, tag="ekv_bf")
            pd = psum1.tile([P, R], F32, tag="pd")
            pn = psum1.tile([P, R], F32, tag="pn")
            for sc in range(SC):
                pkv = psum.tile([P, 2 * HID], F32, tag="pkv")
                for dc in range(DC):
                    nc.tensor.matmul(
                        pkv[:],
                        lhsT=kv(sc, dc),
                        rhs=kvW(dc),
                        start=(dc == 0),
                        stop=(dc == DC - 1),
                    )
                nc.scalar.activation(ek_bf[:, sc, :], pkv[:, :HID], ACT.Exp)
                nc.vector.tensor_mul(ekv_bf[:, sc, :], ek_bf[:, sc, :], pkv[:, HID:])
                nc.tensor.matmul(
                    pd[:],
                    lhsT=ek_bf[:, sc, :],
                    rhs=wT_bf[:, sc, :],
                    start=(sc == 0),
                    stop=(sc == SC - 1),
                )
                nc.tensor.matmul(
                    pn[:],
                    lhsT=ekv_bf[:, sc, :],
                    rhs=wT_bf[:, sc, :],
                    start=(sc == 0),
                    stop=(sc == SC - 1),
                )

            # ---- qhT (h,t); sigmoid via exp so ACT never switches tables:
            # sigmoid(qh) = 1/(1+e) with e = exp(-(qh + qW_b))  (host sends -qW_b)
            pq = psum1.tile([P, R], F32, tag="pq")
            for dc in range(DC):
                nc.tensor.matmul(
                    pq[:],
                    lhsT=qWT(dc),
                    rhs=qTv(dc),
                    start=(dc == 0),
                    stop=(dc == DC - 1),
                )
            e_sb = persist.tile([P, R], F32, tag="e_sb")
            nc.scalar.activation(e_sb[:], pq[:], ACT.Exp, bias=qb, scale=-1.0)

            # ---- out = (num + vb*den) / ((1+e)*den) ----
            # vb*den on ACT (Copy with per-partition scale) so no DVE op reads
            # two PSUM tensors at once.
            vbd_sb = persist.tile([P, R], F32, tag="vbd_sb")
            nc.scalar.mul(vbd_sb[:], pd[:], vb)
            t1_sb = persist.tile([P, R], F32, tag="t1_sb")
            nc.vector.scalar_tensor_tensor(
                t1_sb[:], e_sb[:], 1.0, pd[:], mybir.AluOpType.add, mybir.AluOpType.mult
            )
            t2_sb = persist.tile([P, R], F32, tag="t2_sb")
            nc.vector.tensor_add(t2_sb[:], vbd_sb[:], pn[:])
            rec_sb = persist.tile([P, R], F32, tag="rec_sb")
            nc.vector.reciprocal_approx_fast(rec_sb[:], t1_sb[:])
            res_sb = persist.tile([P, R], F32, tag="res_sb")
            nc.vector.tensor_mul(res_sb[:], t2_sb[:], rec_sb[:])
            nc.sync.dma_start(out=out[:], in_=res_sb[:])

    nc.finalize()
    return nc


_NC_CACHE = None


def _get_nc():
    global _NC_CACHE
    if _NC_CACHE is None:
        _NC_CACHE = _build()
    return _NC_CACHE


def _make_in_maps(q, kv, qW_w, qW_b, kW_w, kW_b, vW_w, vW_b, w_bias_u, w_bias_v):
    f = lambda a: np.ascontiguousarray(np.asarray(a, dtype=np.float32))
    q, kv = f(q), f(kv)
    kvW = np.concatenate([np.asarray(kW_w), np.asarray(vW_w)], axis=0)  # (2H, DIM)
    # kvWT tiled (P, DC, 2H): [p, dc, n] = kvW[n, dc*P+p]
    kvWT_t = np.transpose(kvW.reshape(2 * HID, DC, P), (2, 1, 0))  # (P, DC, 2H)
    qWT_t = np.transpose(np.asarray(qW_w).reshape(HID, DC, P), (2, 1, 0))  # (P,DC,H)
    wbv = np.asarray(w_bias_v)  # (BDIM, TKV)
    u = np.asarray(w_bias_u)
    qf = q.reshape(B * TQ, DIM)
    # kv[b] tiled (P, 2, DC, TKV//2): [p, sh, dc, sw] = kv[b, sh*256+sw, dc*P+p]
    kv_t = [
        np.transpose(kv[b].reshape(2, TKV // 2, DC, P), (3, 0, 2, 1)) for b in range(B)
    ]
    slab1_shared = np.concatenate(
        [kvWT_t.reshape(P, -1), wbv, np.zeros((P, R), np.float32)], axis=1
    )
    in_maps = []
    for i in range(N_CORES):
        b = i // (N_CORES // B)
        t0 = (i % (N_CORES // B)) * R
        s1 = slab1_shared.copy()
        s1[:, O_UT : O_UT + R] = u[t0 : t0 + R].T  # (BDIM, R)
        # qT tiled: [p, dc, t] = qf[i*R + t, dc*P+p]
        qT_t = np.transpose(
            qf[i * R : (i + 1) * R].reshape(R, DC, P), (2, 1, 0)
        )  # (P, DC, R)
        s4 = np.concatenate(
            [
                qWT_t.reshape(P, -1),
                qT_t.reshape(P, -1),
                -np.asarray(qW_b, np.float32).reshape(P, 1),
                np.asarray(vW_b, np.float32).reshape(P, 1),
            ],
            axis=1,
        )
        in_maps.append(
            {
                "s1": f(s1),
                "s2": f(kv_t[b][:, 0].reshape(P, -1)),
                "s3": f(kv_t[b][:, 1].reshape(P, -1)),
                "s4": f(s4),
            }
        )
    return in_maps


def _run(in_maps, trace=False):
    nc = _get_nc()
    return run_bass_kernel_spmd(
        nc, in_maps, core_ids=list(range(N_CORES)), trace=trace
    )


def kernel(**inputs) -> np.ndarray:
    in_maps = _make_in_maps(**inputs)
    res = _run(in_maps)
    out = np.empty((B * TQ, HID), dtype=np.float32)
    for i in range(N_CORES):
        out[i * R : (i + 1) * R] = res.results[i]["out"].T
    return out.reshape(B, TQ, HID)
